# revision 7
# baseline (speedup 1.0000x reference)
"""Trainium2 Bass kernel for nn_DifferentiableAggregation (segment_reduce).

Computes, for batch of 8192 segments over 8388608 sub-images:
    s0[g]  = sum over i with idx_i == g of sub_logits[i, 0]
    s12[g] = sum over i with idx_i == g of (sub_logits[i, 1] + sub_logits[i, 2])
    out[g] = [log(sigmoid(10*(1-s12[g])) + 1e-10),
              log(sigmoid(10*(5-s0[g]))  + 1e-10)]

Strategy: shard the sub-image axis across 8 NeuronCores. Each core does a
local segment-sum via one-hot matmuls accumulating in PSUM (index split as
g = r*64 + q with r = idx>>6 on the 128 PSUM partitions and q = idx&63 in
the free dim), then an AllReduce of the [128, 128] partial and the
sigmoid/log epilogue.

v3 performance design (measured on HW):
  - matmul operands bf16 and CONTIGUOUS: back-to-back 128x128x128 bf16
    matmuls with changing weights run at ~59 ns on HW when lhsT/rhs slices
    are contiguous, vs ~255 ns when strided (4.3x). So one-hots stay in
    the [P, j, k] layout whose per-group k-slices are packed.
  - DVE 16-bit 2x mode requires every operand's innermost AP dim to be
    packed (stride 1, count >= 2). A broadcast along k (stride-0 last dim)
    disqualifies. Trick: the host sends r/q index arrays DUPLICATED pairwise
    (rt2[p, 2t] = rt2[p, 2t+1] = r[p, t]) and the channel values are
    duplicated on device, so every "broadcast" operand can be expressed
    with innermost dim [stride 1, count 2] over the duplicate pair, with
    the 64/32-wide k-half broadcast moved to a middle (stride-0) dim.
    Every one-hot/multiply instruction then runs in the 2x DVE mode.
  - DVE is then the sole bottleneck (~96% busy). For a fraction of blocks
    the value-weighted q one-hot (VQ) is built by the GPSIMD engine's
    local_scatter instruction instead (scatter channel values to
    j*64+q against a zeroed destination), which skips both the q one-hot
    compare and the multiply on DVE for those blocks and overlaps with
    DVE building the r one-hots.
"""

import sys

sys.path.insert(0, "/opt/trn_rl_repo")

import numpy as np
import ml_dtypes

from concourse import bass, bacc, mybir, tile
from concourse.bass_utils import run_bass_kernel_spmd

N_CORES = 8
TOTAL = 8388608
SHARD = TOTAL // N_CORES  # 1048576
BATCH = 8192
P = 128
F32 = mybir.dt.float32
BF16 = mybir.dt.bfloat16
NP_BF16 = ml_dtypes.bfloat16

K_SHARP = 10.0
EPS = 1e-10

S = 64   # element-groups per one-hot build block
SS = 16  # groups per GPSIMD local_scatter call (num_elems = SS*64 <= 2047)


def is_gp_block(blk):
    """Blocks whose VQ is built by GPSIMD local_scatter instead of DVE.

    Measured on HW: a GP block costs ~17.6us of GPSIMD time (the scatter
    zeroes its destination, which dominates) vs ~6.9us of DVE time saved,
    so the balance point is ~46% of blocks on GPSIMD."""
    return blk % 13 < 6


def build_nc(to_count, ti):
    """Build + compile the SPMD bass program. Per core handles
    to_count * 128 * ti elements."""
    shard = to_count * P * ti
    assert shard == SHARD
    nc = bacc.Bacc(
        "TRN2",
        debug=False,
        target_bir_lowering=False,
        num_devices=N_CORES,
    )
    v0_in = nc.dram_tensor("v0", [shard], F32, kind="ExternalInput")
    v1_in = nc.dram_tensor("v1", [shard], F32, kind="ExternalInput")
    v2_in = nc.dram_tensor("v2", [shard], F32, kind="ExternalInput")
    r_in = nc.dram_tensor("ridx2", [2 * shard], BF16, kind="ExternalInput")
    q_in = nc.dram_tensor("qidx2", [2 * shard], BF16, kind="ExternalInput")
    s_in = nc.dram_tensor("spos", [shard], mybir.dt.int16, kind="ExternalInput")
    io128_in = nc.dram_tensor("iota128", [P, 128], BF16, kind="ExternalInput")
    io64_in = nc.dram_tensor("iota64", [P, 64], BF16, kind="ExternalInput")
    out_logits = nc.dram_tensor("logits", [2, BATCH], F32, kind="ExternalOutput")

    with tile.TileContext(nc) as tc:
        _kernel_body(
            tc, to_count, ti,
            v0_in, v1_in, v2_in, r_in, q_in, s_in, io128_in, io64_in,
            out_logits,
        )
    nc.compile()
    return nc


def _kernel_body(tc, to_count, ti,
                 v0_in, v1_in, v2_in, r_in, q_in, s_in, io128_in, io64_in,
                 out_logits):
    nc = tc.nc
    add = mybir.AluOpType.add
    is_equal = mybir.AluOpType.is_equal
    mult = mybir.AluOpType.mult
    AF = mybir.ActivationFunctionType

    v0v = v0_in.ap().rearrange("(o p f) -> o p f", p=P, f=ti)
    v1v = v1_in.ap().rearrange("(o p f) -> o p f", p=P, f=ti)
    v2v = v2_in.ap().rearrange("(o p f) -> o p f", p=P, f=ti)
    rv = r_in.ap().rearrange("(o p f) -> o p f", p=P, f=2 * ti)
    qv = q_in.ap().rearrange("(o p f) -> o p f", p=P, f=2 * ti)
    sv = s_in.ap().rearrange("(o p f) -> o p f", p=P, f=ti)

    assert ti % S == 0
    nb = ti // S
    nblocks = to_count * nb

    with (
        tc.tile_pool(name="const", bufs=1) as cpool,
        tc.tile_pool(name="data", bufs=2) as dpool,
        tc.tile_pool(name="bpool", bufs=2) as bpool,
        tc.tile_pool(name="ohq", bufs=2) as opool,
        tc.tile_pool(name="vq", bufs=2) as vpool,
        tc.tile_pool(name="psum", bufs=1, space="PSUM") as ppool,
        tc.tile_pool(name="epi", bufs=1) as epool,
        tc.tile_pool(name="dram", bufs=1, space="DRAM") as drampool,
    ):
        io128 = cpool.tile([P, 128], BF16)
        nc.sync.dma_start(io128[:], io128_in.ap())
        io64 = cpool.tile([P, 64], BF16)
        nc.sync.dma_start(io64[:], io64_in.ap())
        # pair views of the iota rows: address k = 2*kh + kl
        io128p = io128[:].rearrange("p (kh kl) -> p kh kl", kl=2)
        io64p = io64[:].rearrange("p (kh kl) -> p kh kl", kl=2)

        acc_e = ppool.tile([P, 128], F32, tag="acc_e")
        acc_o = ppool.tile([P, 128], F32, tag="acc_o")

        for to in range(to_count):
            vt0 = dpool.tile([P, ti], F32, tag="vt0")
            nc.sync.dma_start(vt0[:], v0v[to])
            vt1 = dpool.tile([P, ti], F32, tag="vt1")
            nc.sync.dma_start(vt1[:], v1v[to])
            vt2 = dpool.tile([P, ti], F32, tag="vt2")
            nc.sync.dma_start(vt2[:], v2v[to])
            rt2 = dpool.tile([P, 2 * ti], BF16, tag="rt2")
            nc.sync.dma_start(rt2[:], rv[to])
            qt2 = dpool.tile([P, 2 * ti], BF16, tag="qt2")
            nc.sync.dma_start(qt2[:], qv[to])
            spt = dpool.tile([P, ti], mybir.dt.int16, tag="spt")
            nc.sync.dma_start(spt[:], sv[to])

            # channel values duplicated pairwise: cpair2[p, c, t, d] = c_c[p, t]
            cpair2 = dpool.tile([P, 2 * 2 * ti], BF16, tag="cpair2")
            cp4 = cpair2[:].rearrange("p (c t d) -> p c t d", c=2, d=2)
            vt0b = (
                vt0[:].rearrange("p (t o) -> p t o", o=1).to_broadcast([P, ti, 2])
            )
            nc.vector.tensor_copy(cp4[:, 0], vt0b)
            vt1b = (
                vt1[:].rearrange("p (t o) -> p t o", o=1).to_broadcast([P, ti, 2])
            )
            vt2b = (
                vt2[:].rearrange("p (t o) -> p t o", o=1).to_broadcast([P, ti, 2])
            )
            nc.vector.tensor_tensor(cp4[:, 1], vt1b, vt2b, add)

            # contiguous (non-duplicated) channel values for local_scatter
            cflat = dpool.tile([P, 2 * ti], BF16, tag="cflat")
            cf2 = cflat[:].rearrange("p (c t) -> p c t", c=2)
            nc.vector.tensor_copy(cf2[:, 0], vt0[:])
            nc.vector.tensor_tensor(cf2[:, 1], vt1[:], vt2[:], add)

            rt2v = rt2[:].rearrange("p (t d) -> p t d", d=2)
            qt2v = qt2[:].rearrange("p (t d) -> p t d", d=2)

            for b in range(nb):
                sl = slice(b * S, (b + 1) * S)
                blk = to * nb + b

                # r one-hot, normal layout: B[p, j, k] = (r[p, j] == k).
                # All operands end in the packed duplicate-pair dim [1, 2]:
                #   in0 = rt2 pairs broadcast over kh (middle stride 0)
                #   in1 = io128 pair view broadcast over j
                B_all = bpool.tile([P, S * 128], BF16, tag="B")
                B4 = B_all[:].rearrange("p (j kh kl) -> p j kh kl", kh=64, kl=2)
                rb = (
                    rt2v[:, sl]
                    .rearrange("p j (o d) -> p j o d", o=1)
                    .to_broadcast([P, S, 64, 2])
                )
                iob = (
                    io128p[:]
                    .rearrange("p (o kh) kl -> p o kh kl", o=1)
                    .to_broadcast([P, S, 64, 2])
                )
                nc.vector.tensor_tensor(B4, rb, iob, is_equal)

                VQ_all = vpool.tile([P, 2 * S * 64], BF16, tag="VQ")
                VQ4 = VQ_all[:].rearrange("p (c j k) -> p c j k", c=2, k=64)
                if is_gp_block(blk):
                    # GPSIMD path: VQ[p, ch, j, q_j] = c_ch[p, j], rest zeroed
                    # by the scatter itself. SS groups per call.
                    for ch in range(2):
                        for m in range(S // SS):
                            jlo = b * S + m * SS
                            nc.gpsimd.local_scatter(
                                VQ4[:, ch, m * SS:(m + 1) * SS],
                                cf2[:, ch, jlo:jlo + SS],
                                spt[:, jlo:jlo + SS],
                                channels=P,
                                num_elems=SS * 64,
                                num_idxs=SS,
                            )
                else:
                    # DVE path: q one-hot then per-channel multiply
                    OHQ_all = opool.tile([P, S * 64], BF16, tag="OHQ")
                    OHQ4 = OHQ_all[:].rearrange(
                        "p (j kh kl) -> p j kh kl", kh=32, kl=2
                    )
                    qb = (
                        qt2v[:, sl]
                        .rearrange("p j (o d) -> p j o d", o=1)
                        .to_broadcast([P, S, 32, 2])
                    )
                    ioqb = (
                        io64p[:]
                        .rearrange("p (o kh) kl -> p o kh kl", o=1)
                        .to_broadcast([P, S, 32, 2])
                    )
                    nc.vector.tensor_tensor(OHQ4, qb, ioqb, is_equal)

                    ohq_pair = OHQ_all[:].rearrange(
                        "p (j kh kl) -> p j kh kl", kh=32, kl=2
                    )
                    for ch in range(2):
                        cb = (
                            cp4[:, ch, sl]
                            .rearrange("p j (o d) -> p j o d", o=1)
                            .to_broadcast([P, S, 32, 2])
                        )
                        vq_ch = VQ4[:, ch].rearrange(
                            "p j (kh kl) -> p j kh kl", kl=2
                        )
                        nc.vector.tensor_tensor(vq_ch, ohq_pair, cb, mult)

                pacc = acc_e if blk % 2 == 0 else acc_o
                Bj = B_all[:].rearrange("p (j k) -> p j k", k=128)
                VQr = VQ_all[:].rearrange("p (c j k) -> p j c k", c=2, k=64)
                for j in range(S):
                    first = blk < 2 and j == 0
                    last = blk >= nblocks - 2 and j == S - 1
                    nc.tensor.matmul(
                        pacc[:],
                        lhsT=Bj[:, j],
                        rhs=VQr[:, j],
                        start=first,
                        stop=last,
                    )

        # Drain both PSUM accumulators (DVE may read only one PSUM input)
        s_sb = epool.tile([P, 128], F32)
        s_o = epool.tile([P, 128], F32)
        nc.vector.tensor_copy(s_o[:], acc_o[:])
        nc.vector.tensor_tensor(s_sb[:], s_o[:], acc_e[:], add)

        # AllReduce partials across the 8 cores (DRAM bounce buffers)
        din = drampool.tile([P, 128], F32)
        dout = drampool.tile([P, 128], F32)
        nc.gpsimd.dma_start(din[:], s_sb[:])
        nc.gpsimd.collective_compute(
            "AllReduce",
            add,
            replica_groups=[list(range(N_CORES))],
            ins=[din.opt()],
            outs=[dout.opt()],
        )
        sf = epool.tile([P, 128], F32)
        nc.gpsimd.dma_start(sf[:], dout[:])

        # Epilogue: out_c = log(sigmoid(z) + eps), z = -10*s + bias_c.
        # sigmoid computed exactly as 1/(1 + exp(-z)) (ACT exp table +
        # accurate DVE reciprocal); -z clamped at 88 to avoid exp
        # overflow (beyond that sigmoid+eps == eps in fp32 anyway).
        beps = epool.tile([P, 1], F32)
        nc.vector.memset(beps[:], EPS)

        def logsig(out_ap, s_ap, zbias):
            mz = epool.tile([P, 64], F32, tag="mz")
            nc.vector.tensor_scalar(mz[:], s_ap, K_SHARP, -zbias,
                                    mybir.AluOpType.mult, mybir.AluOpType.add)
            nc.vector.tensor_scalar(mz[:], mz[:], 88.0, None,
                                    mybir.AluOpType.min)
            w = epool.tile([P, 64], F32, tag="w")
            nc.scalar.activation(w[:], mz[:], AF.Exp, bias=0.0, scale=1.0)
            nc.vector.tensor_scalar(w[:], w[:], 1.0, None,
                                    mybir.AluOpType.add)
            r = epool.tile([P, 64], F32, tag="r")
            nc.vector.reciprocal(r[:], w[:])
            nc.scalar.activation(out_ap, r[:], AF.Ln, bias=beps[:], scale=1.0)

        o1 = epool.tile([P, 64], F32)
        logsig(o1[:], sf[:, 64:128], K_SHARP)
        o0 = epool.tile([P, 64], F32)
        logsig(o0[:], sf[:, 0:64], 5.0 * K_SHARP)

        ol = out_logits.ap().rearrange("w (p t) -> w p t", p=P, t=BATCH // P)
        nc.sync.dma_start(ol[0], o1[:])
        nc.sync.dma_start(ol[1], o0[:])


_NC_CACHE = {}


def _get_nc(to_count, ti):
    key = (to_count, ti)
    if key not in _NC_CACHE:
        _NC_CACHE[key] = build_nc(to_count, ti)
    return _NC_CACHE[key]


def make_in_maps(sub_logits, original_indices, to_count, ti):
    shard = to_count * P * ti
    idx = np.asarray(original_indices).astype(np.int32)
    v = np.asarray(sub_logits, dtype=np.float32)
    # duplicated pairwise so DVE broadcast reads end in a packed [1, 2] dim
    r2 = np.repeat((idx >> 6).astype(NP_BF16), 2)
    q2 = np.repeat((idx & 63).astype(NP_BF16), 2)
    v0 = np.ascontiguousarray(v[:, 0]).reshape(N_CORES, shard)
    v1 = np.ascontiguousarray(v[:, 1]).reshape(N_CORES, shard)
    v2 = np.ascontiguousarray(v[:, 2]).reshape(N_CORES, shard)
    rs = r2.reshape(N_CORES, 2 * shard)
    qs = q2.reshape(N_CORES, 2 * shard)
    io128 = np.ascontiguousarray(
        np.broadcast_to(np.arange(128, dtype=NP_BF16), (P, 128))
    )
    io64 = np.ascontiguousarray(
        np.broadcast_to(np.arange(64, dtype=NP_BF16), (P, 64))
    )
    # local_scatter positions: element at column t scatters to
    # (t % SS) * 64 + q within its SS-group window
    tmod = ((np.arange(ti, dtype=np.int16) % SS) * 64)
    spos = ((idx & 63).astype(np.int16).reshape(N_CORES, to_count, P, ti)
            + tmod[None, None, None, :])
    spos = np.ascontiguousarray(spos).reshape(N_CORES, shard)
    return [
        {
            "v0": v0[c],
            "v1": v1[c],
            "v2": v2[c],
            "ridx2": rs[c],
            "qidx2": qs[c],
            "spos": spos[c],
            "iota128": io128,
            "iota64": io64,
        }
        for c in range(N_CORES)
    ]


def kernel(sub_logits, original_indices, batch_size=None, _trace=False):
    to_count, ti = 16, 512
    nc = _get_nc(to_count, ti)
    in_maps = make_in_maps(sub_logits, original_indices, to_count, ti)
    res = run_bass_kernel_spmd(
        nc, in_maps, core_ids=list(range(N_CORES)), trace=_trace
    )
    logits = res.results[0]["logits"]
    out = np.stack([logits[0], logits[1]], axis=1).astype(np.float32)
    if _trace:
        kernel._last_results = res
    return out


# revision 8
# speedup vs baseline: 1.1050x; 1.1050x over previous
"""Trainium2 Bass kernel for nn_DifferentiableAggregation (segment_reduce).

Computes, for batch of 8192 segments over 8388608 sub-images:
    s0[g]  = sum over i with idx_i == g of sub_logits[i, 0]
    s12[g] = sum over i with idx_i == g of (sub_logits[i, 1] + sub_logits[i, 2])
    out[g] = [log(sigmoid(10*(1-s12[g])) + 1e-10),
              log(sigmoid(10*(5-s0[g]))  + 1e-10)]

Strategy: shard the sub-image axis across 8 NeuronCores. Each core does a
local segment-sum via one-hot matmuls accumulating in PSUM (index split as
g = r*64 + q with r = idx>>6 on the 128 PSUM partitions and q = idx&63 in
the free dim), then an AllReduce of the [128, 128] partial and the
sigmoid/log epilogue.

v3 performance design (measured on HW):
  - matmul operands bf16 and CONTIGUOUS: back-to-back 128x128x128 bf16
    matmuls with changing weights run at ~59 ns on HW when lhsT/rhs slices
    are contiguous, vs ~255 ns when strided (4.3x). So one-hots stay in
    the [P, j, k] layout whose per-group k-slices are packed.
  - DVE 16-bit 2x mode requires every operand's innermost AP dim to be
    packed (stride 1, count >= 2). A broadcast along k (stride-0 last dim)
    disqualifies. Trick: the host sends r/q index arrays DUPLICATED pairwise
    (rt2[p, 2t] = rt2[p, 2t+1] = r[p, t]) and the channel values are
    duplicated on device, so every "broadcast" operand can be expressed
    with innermost dim [stride 1, count 2] over the duplicate pair, with
    the 64/32-wide k-half broadcast moved to a middle (stride-0) dim.
    Every one-hot/multiply instruction then runs in the 2x DVE mode.
  - DVE is then the sole bottleneck (~96% busy). For a fraction of blocks
    the value-weighted q one-hot (VQ) is built by the GPSIMD engine's
    local_scatter instruction instead (scatter channel values to
    j*64+q against a zeroed destination), which skips both the q one-hot
    compare and the multiply on DVE for those blocks and overlaps with
    DVE building the r one-hots.
"""

import sys

sys.path.insert(0, "/opt/trn_rl_repo")

import numpy as np
import ml_dtypes

from concourse import bass, bacc, mybir, tile
from concourse.bass_utils import run_bass_kernel_spmd

N_CORES = 8
TOTAL = 8388608
SHARD = TOTAL // N_CORES  # 1048576
BATCH = 8192
P = 128
F32 = mybir.dt.float32
BF16 = mybir.dt.bfloat16
NP_BF16 = ml_dtypes.bfloat16

K_SHARP = 10.0
EPS = 1e-10

S = 64   # element-groups per one-hot build block
SS = 16  # groups per GPSIMD local_scatter call (num_elems = SS*64 <= 2047)


def is_gp_block(blk):
    """Blocks whose VQ is built by GPSIMD local_scatter instead of DVE.

    Measured on HW: a GP block costs ~17.6us of GPSIMD time (the scatter
    zeroes its destination, which dominates) vs ~6.9us of DVE time saved,
    so the balance point is ~46% of blocks on GPSIMD. GP blocks must be
    interleaved as singletons — consecutive GP blocks overrun the bufs=2
    pool pipelining and stall DVE (measured: clustered 6-in-13 pattern ran
    120us slower than alternating)."""
    return blk % 13 in (0, 2, 4, 6, 8, 10)


def build_nc(to_count, ti):
    """Build + compile the SPMD bass program. Per core handles
    to_count * 128 * ti elements."""
    shard = to_count * P * ti
    assert shard == SHARD
    nc = bacc.Bacc(
        "TRN2",
        debug=False,
        target_bir_lowering=False,
        num_devices=N_CORES,
    )
    v0_in = nc.dram_tensor("v0", [shard], F32, kind="ExternalInput")
    v1_in = nc.dram_tensor("v1", [shard], F32, kind="ExternalInput")
    v2_in = nc.dram_tensor("v2", [shard], F32, kind="ExternalInput")
    r_in = nc.dram_tensor("ridx2", [2 * shard], BF16, kind="ExternalInput")
    q_in = nc.dram_tensor("qidx2", [2 * shard], BF16, kind="ExternalInput")
    s_in = nc.dram_tensor("spos", [shard], mybir.dt.int16, kind="ExternalInput")
    io128_in = nc.dram_tensor("iota128", [P, 128], BF16, kind="ExternalInput")
    io64_in = nc.dram_tensor("iota64", [P, 64], BF16, kind="ExternalInput")
    out_logits = nc.dram_tensor("logits", [2, BATCH], F32, kind="ExternalOutput")

    with tile.TileContext(nc) as tc:
        _kernel_body(
            tc, to_count, ti,
            v0_in, v1_in, v2_in, r_in, q_in, s_in, io128_in, io64_in,
            out_logits,
        )
    nc.compile()
    return nc


def _kernel_body(tc, to_count, ti,
                 v0_in, v1_in, v2_in, r_in, q_in, s_in, io128_in, io64_in,
                 out_logits):
    nc = tc.nc
    add = mybir.AluOpType.add
    is_equal = mybir.AluOpType.is_equal
    mult = mybir.AluOpType.mult
    AF = mybir.ActivationFunctionType

    v0v = v0_in.ap().rearrange("(o p f) -> o p f", p=P, f=ti)
    v1v = v1_in.ap().rearrange("(o p f) -> o p f", p=P, f=ti)
    v2v = v2_in.ap().rearrange("(o p f) -> o p f", p=P, f=ti)
    rv = r_in.ap().rearrange("(o p f) -> o p f", p=P, f=2 * ti)
    qv = q_in.ap().rearrange("(o p f) -> o p f", p=P, f=2 * ti)
    sv = s_in.ap().rearrange("(o p f) -> o p f", p=P, f=ti)

    assert ti % S == 0
    nb = ti // S
    nblocks = to_count * nb

    with (
        tc.tile_pool(name="const", bufs=1) as cpool,
        tc.tile_pool(name="data", bufs=2) as dpool,
        tc.tile_pool(name="bpool", bufs=2) as bpool,
        tc.tile_pool(name="ohq", bufs=2) as opool,
        tc.tile_pool(name="vq", bufs=2) as vpool,
        tc.tile_pool(name="psum", bufs=1, space="PSUM") as ppool,
        tc.tile_pool(name="epi", bufs=1) as epool,
        tc.tile_pool(name="dram", bufs=1, space="DRAM") as drampool,
    ):
        io128 = cpool.tile([P, 128], BF16)
        nc.sync.dma_start(io128[:], io128_in.ap())
        io64 = cpool.tile([P, 64], BF16)
        nc.sync.dma_start(io64[:], io64_in.ap())
        # pair views of the iota rows: address k = 2*kh + kl
        io128p = io128[:].rearrange("p (kh kl) -> p kh kl", kl=2)
        io64p = io64[:].rearrange("p (kh kl) -> p kh kl", kl=2)

        acc_e = ppool.tile([P, 128], F32, tag="acc_e")
        acc_o = ppool.tile([P, 128], F32, tag="acc_o")

        for to in range(to_count):
            vt0 = dpool.tile([P, ti], F32, tag="vt0")
            nc.sync.dma_start(vt0[:], v0v[to])
            vt1 = dpool.tile([P, ti], F32, tag="vt1")
            nc.sync.dma_start(vt1[:], v1v[to])
            vt2 = dpool.tile([P, ti], F32, tag="vt2")
            nc.sync.dma_start(vt2[:], v2v[to])
            rt2 = dpool.tile([P, 2 * ti], BF16, tag="rt2")
            nc.sync.dma_start(rt2[:], rv[to])
            qt2 = dpool.tile([P, 2 * ti], BF16, tag="qt2")
            nc.sync.dma_start(qt2[:], qv[to])
            spt = dpool.tile([P, ti], mybir.dt.int16, tag="spt")
            nc.sync.dma_start(spt[:], sv[to])

            # channel values duplicated pairwise: cpair2[p, c, t, d] = c_c[p, t]
            cpair2 = dpool.tile([P, 2 * 2 * ti], BF16, tag="cpair2")
            cp4 = cpair2[:].rearrange("p (c t d) -> p c t d", c=2, d=2)
            vt0b = (
                vt0[:].rearrange("p (t o) -> p t o", o=1).to_broadcast([P, ti, 2])
            )
            nc.vector.tensor_copy(cp4[:, 0], vt0b)
            vt1b = (
                vt1[:].rearrange("p (t o) -> p t o", o=1).to_broadcast([P, ti, 2])
            )
            vt2b = (
                vt2[:].rearrange("p (t o) -> p t o", o=1).to_broadcast([P, ti, 2])
            )
            nc.vector.tensor_tensor(cp4[:, 1], vt1b, vt2b, add)

            # contiguous (non-duplicated) channel values for local_scatter
            cflat = dpool.tile([P, 2 * ti], BF16, tag="cflat")
            cf2 = cflat[:].rearrange("p (c t) -> p c t", c=2)
            nc.vector.tensor_copy(cf2[:, 0], vt0[:])
            nc.vector.tensor_tensor(cf2[:, 1], vt1[:], vt2[:], add)

            rt2v = rt2[:].rearrange("p (t d) -> p t d", d=2)
            qt2v = qt2[:].rearrange("p (t d) -> p t d", d=2)

            for b in range(nb):
                sl = slice(b * S, (b + 1) * S)
                blk = to * nb + b

                # r one-hot, normal layout: B[p, j, k] = (r[p, j] == k).
                # All operands end in the packed duplicate-pair dim [1, 2]:
                #   in0 = rt2 pairs broadcast over kh (middle stride 0)
                #   in1 = io128 pair view broadcast over j
                B_all = bpool.tile([P, S * 128], BF16, tag="B")
                B4 = B_all[:].rearrange("p (j kh kl) -> p j kh kl", kh=64, kl=2)
                rb = (
                    rt2v[:, sl]
                    .rearrange("p j (o d) -> p j o d", o=1)
                    .to_broadcast([P, S, 64, 2])
                )
                iob = (
                    io128p[:]
                    .rearrange("p (o kh) kl -> p o kh kl", o=1)
                    .to_broadcast([P, S, 64, 2])
                )
                nc.vector.tensor_tensor(B4, rb, iob, is_equal)

                VQ_all = vpool.tile([P, 2 * S * 64], BF16, tag="VQ")
                VQ4 = VQ_all[:].rearrange("p (c j k) -> p c j k", c=2, k=64)
                if is_gp_block(blk):
                    # GPSIMD path: VQ[p, ch, j, q_j] = c_ch[p, j], rest zeroed
                    # by the scatter itself. SS groups per call.
                    for ch in range(2):
                        for m in range(S // SS):
                            jlo = b * S + m * SS
                            nc.gpsimd.local_scatter(
                                VQ4[:, ch, m * SS:(m + 1) * SS],
                                cf2[:, ch, jlo:jlo + SS],
                                spt[:, jlo:jlo + SS],
                                channels=P,
                                num_elems=SS * 64,
                                num_idxs=SS,
                            )
                else:
                    # DVE path: q one-hot then per-channel multiply
                    OHQ_all = opool.tile([P, S * 64], BF16, tag="OHQ")
                    OHQ4 = OHQ_all[:].rearrange(
                        "p (j kh kl) -> p j kh kl", kh=32, kl=2
                    )
                    qb = (
                        qt2v[:, sl]
                        .rearrange("p j (o d) -> p j o d", o=1)
                        .to_broadcast([P, S, 32, 2])
                    )
                    ioqb = (
                        io64p[:]
                        .rearrange("p (o kh) kl -> p o kh kl", o=1)
                        .to_broadcast([P, S, 32, 2])
                    )
                    nc.vector.tensor_tensor(OHQ4, qb, ioqb, is_equal)

                    ohq_pair = OHQ_all[:].rearrange(
                        "p (j kh kl) -> p j kh kl", kh=32, kl=2
                    )
                    for ch in range(2):
                        cb = (
                            cp4[:, ch, sl]
                            .rearrange("p j (o d) -> p j o d", o=1)
                            .to_broadcast([P, S, 32, 2])
                        )
                        vq_ch = VQ4[:, ch].rearrange(
                            "p j (kh kl) -> p j kh kl", kl=2
                        )
                        nc.vector.tensor_tensor(vq_ch, ohq_pair, cb, mult)

                pacc = acc_e if blk % 2 == 0 else acc_o
                Bj = B_all[:].rearrange("p (j k) -> p j k", k=128)
                VQr = VQ_all[:].rearrange("p (c j k) -> p j c k", c=2, k=64)
                for j in range(S):
                    first = blk < 2 and j == 0
                    last = blk >= nblocks - 2 and j == S - 1
                    nc.tensor.matmul(
                        pacc[:],
                        lhsT=Bj[:, j],
                        rhs=VQr[:, j],
                        start=first,
                        stop=last,
                    )

        # Drain both PSUM accumulators (DVE may read only one PSUM input)
        s_sb = epool.tile([P, 128], F32)
        s_o = epool.tile([P, 128], F32)
        nc.vector.tensor_copy(s_o[:], acc_o[:])
        nc.vector.tensor_tensor(s_sb[:], s_o[:], acc_e[:], add)

        # AllReduce partials across the 8 cores (DRAM bounce buffers)
        din = drampool.tile([P, 128], F32)
        dout = drampool.tile([P, 128], F32)
        nc.gpsimd.dma_start(din[:], s_sb[:])
        nc.gpsimd.collective_compute(
            "AllReduce",
            add,
            replica_groups=[list(range(N_CORES))],
            ins=[din.opt()],
            outs=[dout.opt()],
        )
        sf = epool.tile([P, 128], F32)
        nc.gpsimd.dma_start(sf[:], dout[:])

        # Epilogue: out_c = log(sigmoid(z) + eps), z = -10*s + bias_c.
        # sigmoid computed exactly as 1/(1 + exp(-z)) (ACT exp table +
        # accurate DVE reciprocal); -z clamped at 88 to avoid exp
        # overflow (beyond that sigmoid+eps == eps in fp32 anyway).
        beps = epool.tile([P, 1], F32)
        nc.vector.memset(beps[:], EPS)

        def logsig(out_ap, s_ap, zbias):
            mz = epool.tile([P, 64], F32, tag="mz")
            nc.vector.tensor_scalar(mz[:], s_ap, K_SHARP, -zbias,
                                    mybir.AluOpType.mult, mybir.AluOpType.add)
            nc.vector.tensor_scalar(mz[:], mz[:], 88.0, None,
                                    mybir.AluOpType.min)
            w = epool.tile([P, 64], F32, tag="w")
            nc.scalar.activation(w[:], mz[:], AF.Exp, bias=0.0, scale=1.0)
            nc.vector.tensor_scalar(w[:], w[:], 1.0, None,
                                    mybir.AluOpType.add)
            r = epool.tile([P, 64], F32, tag="r")
            nc.vector.reciprocal(r[:], w[:])
            nc.scalar.activation(out_ap, r[:], AF.Ln, bias=beps[:], scale=1.0)

        o1 = epool.tile([P, 64], F32)
        logsig(o1[:], sf[:, 64:128], K_SHARP)
        o0 = epool.tile([P, 64], F32)
        logsig(o0[:], sf[:, 0:64], 5.0 * K_SHARP)

        ol = out_logits.ap().rearrange("w (p t) -> w p t", p=P, t=BATCH // P)
        nc.sync.dma_start(ol[0], o1[:])
        nc.sync.dma_start(ol[1], o0[:])


_NC_CACHE = {}


def _get_nc(to_count, ti):
    key = (to_count, ti)
    if key not in _NC_CACHE:
        _NC_CACHE[key] = build_nc(to_count, ti)
    return _NC_CACHE[key]


def make_in_maps(sub_logits, original_indices, to_count, ti):
    shard = to_count * P * ti
    idx = np.asarray(original_indices).astype(np.int32)
    v = np.asarray(sub_logits, dtype=np.float32)
    # duplicated pairwise so DVE broadcast reads end in a packed [1, 2] dim
    r2 = np.repeat((idx >> 6).astype(NP_BF16), 2)
    q2 = np.repeat((idx & 63).astype(NP_BF16), 2)
    v0 = np.ascontiguousarray(v[:, 0]).reshape(N_CORES, shard)
    v1 = np.ascontiguousarray(v[:, 1]).reshape(N_CORES, shard)
    v2 = np.ascontiguousarray(v[:, 2]).reshape(N_CORES, shard)
    rs = r2.reshape(N_CORES, 2 * shard)
    qs = q2.reshape(N_CORES, 2 * shard)
    io128 = np.ascontiguousarray(
        np.broadcast_to(np.arange(128, dtype=NP_BF16), (P, 128))
    )
    io64 = np.ascontiguousarray(
        np.broadcast_to(np.arange(64, dtype=NP_BF16), (P, 64))
    )
    # local_scatter positions: element at column t scatters to
    # (t % SS) * 64 + q within its SS-group window
    tmod = ((np.arange(ti, dtype=np.int16) % SS) * 64)
    spos = ((idx & 63).astype(np.int16).reshape(N_CORES, to_count, P, ti)
            + tmod[None, None, None, :])
    spos = np.ascontiguousarray(spos).reshape(N_CORES, shard)
    return [
        {
            "v0": v0[c],
            "v1": v1[c],
            "v2": v2[c],
            "ridx2": rs[c],
            "qidx2": qs[c],
            "spos": spos[c],
            "iota128": io128,
            "iota64": io64,
        }
        for c in range(N_CORES)
    ]


def kernel(sub_logits, original_indices, batch_size=None, _trace=False):
    to_count, ti = 16, 512
    nc = _get_nc(to_count, ti)
    in_maps = make_in_maps(sub_logits, original_indices, to_count, ti)
    res = run_bass_kernel_spmd(
        nc, in_maps, core_ids=list(range(N_CORES)), trace=_trace
    )
    logits = res.results[0]["logits"]
    out = np.stack([logits[0], logits[1]], axis=1).astype(np.float32)
    if _trace:
        kernel._last_results = res
    return out


# revision 9
# speedup vs baseline: 1.1124x; 1.0067x over previous
"""Trainium2 Bass kernel for nn_DifferentiableAggregation (segment_reduce).

Computes, for batch of 8192 segments over 8388608 sub-images:
    s0[g]  = sum over i with idx_i == g of sub_logits[i, 0]
    s12[g] = sum over i with idx_i == g of (sub_logits[i, 1] + sub_logits[i, 2])
    out[g] = [log(sigmoid(10*(1-s12[g])) + 1e-10),
              log(sigmoid(10*(5-s0[g]))  + 1e-10)]

Strategy: shard the sub-image axis across 8 NeuronCores. Each core does a
local segment-sum via one-hot matmuls accumulating in PSUM (index split as
g = r*64 + q with r = idx>>6 on the 128 PSUM partitions and q = idx&63 in
the free dim), then an AllReduce of the [128, 128] partial and the
sigmoid/log epilogue.

v3 performance design (measured on HW):
  - matmul operands bf16 and CONTIGUOUS: back-to-back 128x128x128 bf16
    matmuls with changing weights run at ~59 ns on HW when lhsT/rhs slices
    are contiguous, vs ~255 ns when strided (4.3x). So one-hots stay in
    the [P, j, k] layout whose per-group k-slices are packed.
  - DVE 16-bit 2x mode requires every operand's innermost AP dim to be
    packed (stride 1, count >= 2). A broadcast along k (stride-0 last dim)
    disqualifies. Trick: the host sends r/q index arrays DUPLICATED pairwise
    (rt2[p, 2t] = rt2[p, 2t+1] = r[p, t]) and the channel values are
    duplicated on device, so every "broadcast" operand can be expressed
    with innermost dim [stride 1, count 2] over the duplicate pair, with
    the 64/32-wide k-half broadcast moved to a middle (stride-0) dim.
    Every one-hot/multiply instruction then runs in the 2x DVE mode.
  - DVE is then the sole bottleneck (~96% busy). For a fraction of blocks
    the value-weighted q one-hot (VQ) is built by the GPSIMD engine's
    local_scatter instruction instead (scatter channel values to
    j*64+q against a zeroed destination), which skips both the q one-hot
    compare and the multiply on DVE for those blocks and overlaps with
    DVE building the r one-hots.
"""

import sys

sys.path.insert(0, "/opt/trn_rl_repo")

import numpy as np
import ml_dtypes

from concourse import bass, bacc, mybir, tile
from concourse.bass_utils import run_bass_kernel_spmd

N_CORES = 8
TOTAL = 8388608
SHARD = TOTAL // N_CORES  # 1048576
BATCH = 8192
P = 128
F32 = mybir.dt.float32
BF16 = mybir.dt.bfloat16
NP_BF16 = ml_dtypes.bfloat16

K_SHARP = 10.0
EPS = 1e-10

S = 64   # element-groups per one-hot build block
SS = 16  # groups per GPSIMD local_scatter call (num_elems = SS*64 <= 2047)


def is_gp_block(blk):
    """Blocks whose VQ is built by GPSIMD local_scatter instead of DVE.

    Measured on HW: a GP block costs ~17.6us of GPSIMD time (the scatter
    zeroes its destination, which dominates) vs ~6.9us of DVE time saved,
    so the balance point is ~46% of blocks on GPSIMD. GP blocks must be
    interleaved as singletons — consecutive GP blocks overrun the bufs=2
    pool pipelining and stall DVE (measured: clustered 6-in-13 pattern ran
    120us slower than alternating)."""
    return blk % 13 in (0, 2, 4, 6, 8, 10)


def build_nc(to_count, ti):
    """Build + compile the SPMD bass program. Per core handles
    to_count * 128 * ti elements."""
    shard = to_count * P * ti
    assert shard == SHARD
    nc = bacc.Bacc(
        "TRN2",
        debug=False,
        target_bir_lowering=False,
        num_devices=N_CORES,
    )
    v0_in = nc.dram_tensor("v0", [shard], F32, kind="ExternalInput")
    v1_in = nc.dram_tensor("v1", [shard], F32, kind="ExternalInput")
    v2_in = nc.dram_tensor("v2", [shard], F32, kind="ExternalInput")
    r_in = nc.dram_tensor("ridx2", [2 * shard], BF16, kind="ExternalInput")
    q_in = nc.dram_tensor("qidx2", [2 * shard], BF16, kind="ExternalInput")
    s_in = nc.dram_tensor("spos", [shard], mybir.dt.int16, kind="ExternalInput")
    io128_in = nc.dram_tensor("iota128", [P, 128], BF16, kind="ExternalInput")
    io64_in = nc.dram_tensor("iota64", [P, 64], BF16, kind="ExternalInput")
    out_logits = nc.dram_tensor("logits", [2, BATCH], F32, kind="ExternalOutput")

    with tile.TileContext(nc) as tc:
        _kernel_body(
            tc, to_count, ti,
            v0_in, v1_in, v2_in, r_in, q_in, s_in, io128_in, io64_in,
            out_logits,
        )
    nc.compile()
    return nc


def _kernel_body(tc, to_count, ti,
                 v0_in, v1_in, v2_in, r_in, q_in, s_in, io128_in, io64_in,
                 out_logits):
    nc = tc.nc
    add = mybir.AluOpType.add
    is_equal = mybir.AluOpType.is_equal
    mult = mybir.AluOpType.mult
    AF = mybir.ActivationFunctionType

    v0v = v0_in.ap().rearrange("(o p f) -> o p f", p=P, f=ti)
    v1v = v1_in.ap().rearrange("(o p f) -> o p f", p=P, f=ti)
    v2v = v2_in.ap().rearrange("(o p f) -> o p f", p=P, f=ti)
    rv = r_in.ap().rearrange("(o p f) -> o p f", p=P, f=2 * ti)
    qv = q_in.ap().rearrange("(o p f) -> o p f", p=P, f=2 * ti)
    sv = s_in.ap().rearrange("(o p f) -> o p f", p=P, f=ti)

    assert ti % S == 0
    nb = ti // S
    nblocks = to_count * nb

    with (
        tc.tile_pool(name="const", bufs=1) as cpool,
        tc.tile_pool(name="data", bufs=2) as dpool,
        tc.tile_pool(name="bpool", bufs=3) as bpool,
        tc.tile_pool(name="ohq", bufs=3) as opool,
        tc.tile_pool(name="vq", bufs=3) as vpool,
        tc.tile_pool(name="psum", bufs=1, space="PSUM") as ppool,
        tc.tile_pool(name="epi", bufs=1) as epool,
        tc.tile_pool(name="dram", bufs=1, space="DRAM") as drampool,
    ):
        io128 = cpool.tile([P, 128], BF16)
        nc.sync.dma_start(io128[:], io128_in.ap())
        io64 = cpool.tile([P, 64], BF16)
        nc.sync.dma_start(io64[:], io64_in.ap())
        # pair views of the iota rows: address k = 2*kh + kl
        io128p = io128[:].rearrange("p (kh kl) -> p kh kl", kl=2)
        io64p = io64[:].rearrange("p (kh kl) -> p kh kl", kl=2)

        acc_e = ppool.tile([P, 128], F32, tag="acc_e")
        acc_o = ppool.tile([P, 128], F32, tag="acc_o")

        for to in range(to_count):
            vt0 = dpool.tile([P, ti], F32, tag="vt0")
            nc.sync.dma_start(vt0[:], v0v[to])
            vt1 = dpool.tile([P, ti], F32, tag="vt1")
            nc.sync.dma_start(vt1[:], v1v[to])
            vt2 = dpool.tile([P, ti], F32, tag="vt2")
            nc.sync.dma_start(vt2[:], v2v[to])
            rt2 = dpool.tile([P, 2 * ti], BF16, tag="rt2")
            nc.sync.dma_start(rt2[:], rv[to])
            qt2 = dpool.tile([P, 2 * ti], BF16, tag="qt2")
            nc.sync.dma_start(qt2[:], qv[to])
            spt = dpool.tile([P, ti], mybir.dt.int16, tag="spt")
            nc.sync.dma_start(spt[:], sv[to])

            # channel values duplicated pairwise: cpair2[p, c, t, d] = c_c[p, t]
            cpair2 = dpool.tile([P, 2 * 2 * ti], BF16, tag="cpair2")
            cp4 = cpair2[:].rearrange("p (c t d) -> p c t d", c=2, d=2)
            vt0b = (
                vt0[:].rearrange("p (t o) -> p t o", o=1).to_broadcast([P, ti, 2])
            )
            nc.vector.tensor_copy(cp4[:, 0], vt0b)
            vt1b = (
                vt1[:].rearrange("p (t o) -> p t o", o=1).to_broadcast([P, ti, 2])
            )
            vt2b = (
                vt2[:].rearrange("p (t o) -> p t o", o=1).to_broadcast([P, ti, 2])
            )
            nc.vector.tensor_tensor(cp4[:, 1], vt1b, vt2b, add)

            # contiguous (non-duplicated) channel values for local_scatter
            cflat = dpool.tile([P, 2 * ti], BF16, tag="cflat")
            cf2 = cflat[:].rearrange("p (c t) -> p c t", c=2)
            nc.vector.tensor_copy(cf2[:, 0], vt0[:])
            nc.vector.tensor_tensor(cf2[:, 1], vt1[:], vt2[:], add)

            rt2v = rt2[:].rearrange("p (t d) -> p t d", d=2)
            qt2v = qt2[:].rearrange("p (t d) -> p t d", d=2)

            for b in range(nb):
                sl = slice(b * S, (b + 1) * S)
                blk = to * nb + b

                # r one-hot, normal layout: B[p, j, k] = (r[p, j] == k).
                # All operands end in the packed duplicate-pair dim [1, 2]:
                #   in0 = rt2 pairs broadcast over kh (middle stride 0)
                #   in1 = io128 pair view broadcast over j
                B_all = bpool.tile([P, S * 128], BF16, tag="B")
                B4 = B_all[:].rearrange("p (j kh kl) -> p j kh kl", kh=64, kl=2)
                rb = (
                    rt2v[:, sl]
                    .rearrange("p j (o d) -> p j o d", o=1)
                    .to_broadcast([P, S, 64, 2])
                )
                iob = (
                    io128p[:]
                    .rearrange("p (o kh) kl -> p o kh kl", o=1)
                    .to_broadcast([P, S, 64, 2])
                )
                nc.vector.tensor_tensor(B4, rb, iob, is_equal)

                VQ_all = vpool.tile([P, 2 * S * 64], BF16, tag="VQ")
                VQ4 = VQ_all[:].rearrange("p (c j k) -> p c j k", c=2, k=64)
                if is_gp_block(blk):
                    # GPSIMD path: VQ[p, ch, j, q_j] = c_ch[p, j], rest zeroed
                    # by the scatter itself. SS groups per call.
                    for ch in range(2):
                        for m in range(S // SS):
                            jlo = b * S + m * SS
                            nc.gpsimd.local_scatter(
                                VQ4[:, ch, m * SS:(m + 1) * SS],
                                cf2[:, ch, jlo:jlo + SS],
                                spt[:, jlo:jlo + SS],
                                channels=P,
                                num_elems=SS * 64,
                                num_idxs=SS,
                            )
                else:
                    # DVE path: q one-hot then per-channel multiply
                    OHQ_all = opool.tile([P, S * 64], BF16, tag="OHQ")
                    OHQ4 = OHQ_all[:].rearrange(
                        "p (j kh kl) -> p j kh kl", kh=32, kl=2
                    )
                    qb = (
                        qt2v[:, sl]
                        .rearrange("p j (o d) -> p j o d", o=1)
                        .to_broadcast([P, S, 32, 2])
                    )
                    ioqb = (
                        io64p[:]
                        .rearrange("p (o kh) kl -> p o kh kl", o=1)
                        .to_broadcast([P, S, 32, 2])
                    )
                    nc.vector.tensor_tensor(OHQ4, qb, ioqb, is_equal)

                    ohq_pair = OHQ_all[:].rearrange(
                        "p (j kh kl) -> p j kh kl", kh=32, kl=2
                    )
                    for ch in range(2):
                        cb = (
                            cp4[:, ch, sl]
                            .rearrange("p j (o d) -> p j o d", o=1)
                            .to_broadcast([P, S, 32, 2])
                        )
                        vq_ch = VQ4[:, ch].rearrange(
                            "p j (kh kl) -> p j kh kl", kl=2
                        )
                        nc.vector.tensor_tensor(vq_ch, ohq_pair, cb, mult)

                pacc = acc_e if blk % 2 == 0 else acc_o
                Bj = B_all[:].rearrange("p (j k) -> p j k", k=128)
                VQr = VQ_all[:].rearrange("p (c j k) -> p j c k", c=2, k=64)
                for j in range(S):
                    first = blk < 2 and j == 0
                    last = blk >= nblocks - 2 and j == S - 1
                    nc.tensor.matmul(
                        pacc[:],
                        lhsT=Bj[:, j],
                        rhs=VQr[:, j],
                        start=first,
                        stop=last,
                    )

        # Drain both PSUM accumulators (DVE may read only one PSUM input)
        s_sb = epool.tile([P, 128], F32)
        s_o = epool.tile([P, 128], F32)
        nc.vector.tensor_copy(s_o[:], acc_o[:])
        nc.vector.tensor_tensor(s_sb[:], s_o[:], acc_e[:], add)

        # AllReduce partials across the 8 cores (DRAM bounce buffers)
        din = drampool.tile([P, 128], F32)
        dout = drampool.tile([P, 128], F32)
        nc.gpsimd.dma_start(din[:], s_sb[:])
        nc.gpsimd.collective_compute(
            "AllReduce",
            add,
            replica_groups=[list(range(N_CORES))],
            ins=[din.opt()],
            outs=[dout.opt()],
        )
        sf = epool.tile([P, 128], F32)
        nc.gpsimd.dma_start(sf[:], dout[:])

        # Epilogue: out_c = log(sigmoid(z) + eps), z = -10*s + bias_c.
        # sigmoid computed exactly as 1/(1 + exp(-z)) (ACT exp table +
        # accurate DVE reciprocal); -z clamped at 88 to avoid exp
        # overflow (beyond that sigmoid+eps == eps in fp32 anyway).
        beps = epool.tile([P, 1], F32)
        nc.vector.memset(beps[:], EPS)

        def logsig(out_ap, s_ap, zbias):
            mz = epool.tile([P, 64], F32, tag="mz")
            nc.vector.tensor_scalar(mz[:], s_ap, K_SHARP, -zbias,
                                    mybir.AluOpType.mult, mybir.AluOpType.add)
            nc.vector.tensor_scalar(mz[:], mz[:], 88.0, None,
                                    mybir.AluOpType.min)
            w = epool.tile([P, 64], F32, tag="w")
            nc.scalar.activation(w[:], mz[:], AF.Exp, bias=0.0, scale=1.0)
            nc.vector.tensor_scalar(w[:], w[:], 1.0, None,
                                    mybir.AluOpType.add)
            r = epool.tile([P, 64], F32, tag="r")
            nc.vector.reciprocal(r[:], w[:])
            nc.scalar.activation(out_ap, r[:], AF.Ln, bias=beps[:], scale=1.0)

        o1 = epool.tile([P, 64], F32)
        logsig(o1[:], sf[:, 64:128], K_SHARP)
        o0 = epool.tile([P, 64], F32)
        logsig(o0[:], sf[:, 0:64], 5.0 * K_SHARP)

        ol = out_logits.ap().rearrange("w (p t) -> w p t", p=P, t=BATCH // P)
        nc.sync.dma_start(ol[0], o1[:])
        nc.sync.dma_start(ol[1], o0[:])


_NC_CACHE = {}


def _get_nc(to_count, ti):
    key = (to_count, ti)
    if key not in _NC_CACHE:
        _NC_CACHE[key] = build_nc(to_count, ti)
    return _NC_CACHE[key]


def make_in_maps(sub_logits, original_indices, to_count, ti):
    shard = to_count * P * ti
    idx = np.asarray(original_indices).astype(np.int32)
    v = np.asarray(sub_logits, dtype=np.float32)
    # duplicated pairwise so DVE broadcast reads end in a packed [1, 2] dim
    r2 = np.repeat((idx >> 6).astype(NP_BF16), 2)
    q2 = np.repeat((idx & 63).astype(NP_BF16), 2)
    v0 = np.ascontiguousarray(v[:, 0]).reshape(N_CORES, shard)
    v1 = np.ascontiguousarray(v[:, 1]).reshape(N_CORES, shard)
    v2 = np.ascontiguousarray(v[:, 2]).reshape(N_CORES, shard)
    rs = r2.reshape(N_CORES, 2 * shard)
    qs = q2.reshape(N_CORES, 2 * shard)
    io128 = np.ascontiguousarray(
        np.broadcast_to(np.arange(128, dtype=NP_BF16), (P, 128))
    )
    io64 = np.ascontiguousarray(
        np.broadcast_to(np.arange(64, dtype=NP_BF16), (P, 64))
    )
    # local_scatter positions: element at column t scatters to
    # (t % SS) * 64 + q within its SS-group window
    tmod = ((np.arange(ti, dtype=np.int16) % SS) * 64)
    spos = ((idx & 63).astype(np.int16).reshape(N_CORES, to_count, P, ti)
            + tmod[None, None, None, :])
    spos = np.ascontiguousarray(spos).reshape(N_CORES, shard)
    return [
        {
            "v0": v0[c],
            "v1": v1[c],
            "v2": v2[c],
            "ridx2": rs[c],
            "qidx2": qs[c],
            "spos": spos[c],
            "iota128": io128,
            "iota64": io64,
        }
        for c in range(N_CORES)
    ]


def kernel(sub_logits, original_indices, batch_size=None, _trace=False):
    to_count, ti = 16, 512
    nc = _get_nc(to_count, ti)
    in_maps = make_in_maps(sub_logits, original_indices, to_count, ti)
    res = run_bass_kernel_spmd(
        nc, in_maps, core_ids=list(range(N_CORES)), trace=_trace
    )
    logits = res.results[0]["logits"]
    out = np.stack([logits[0], logits[1]], axis=1).astype(np.float32)
    if _trace:
        kernel._last_results = res
    return out


# revision 10
# speedup vs baseline: 1.3832x; 1.2434x over previous
"""Trainium2 Bass kernel for nn_DifferentiableAggregation (segment_reduce).

Computes, for batch of 8192 segments over 8388608 sub-images:
    s0[g]  = sum over i with idx_i == g of sub_logits[i, 0]
    s12[g] = sum over i with idx_i == g of (sub_logits[i, 1] + sub_logits[i, 2])
    out[g] = [log(sigmoid(10*(1-s12[g])) + 1e-10),
              log(sigmoid(10*(5-s0[g]))  + 1e-10)]

Strategy: shard the sub-image axis across 8 NeuronCores. Each core does a
local segment-sum via one-hot matmuls accumulating in PSUM (index split as
g = r*64 + q with r = idx>>6 on the 128 PSUM partitions and q = idx&63 in
the free dim), then an AllReduce of the [128, 128] partial and the
sigmoid/log epilogue.

v3 performance design (measured on HW):
  - matmul operands bf16 and CONTIGUOUS: back-to-back 128x128x128 bf16
    matmuls with changing weights run at ~59 ns on HW when lhsT/rhs slices
    are contiguous, vs ~255 ns when strided (4.3x). So one-hots stay in
    the [P, j, k] layout whose per-group k-slices are packed.
  - DVE 16-bit 2x mode requires every operand's innermost AP dim to be
    packed (stride 1, count >= 2). A broadcast along k (stride-0 last dim)
    disqualifies. Trick: the host sends r/q index arrays DUPLICATED pairwise
    (rt2[p, 2t] = rt2[p, 2t+1] = r[p, t]) and the channel values are
    duplicated on device, so every "broadcast" operand can be expressed
    with innermost dim [stride 1, count 2] over the duplicate pair, with
    the 64/32-wide k-half broadcast moved to a middle (stride-0) dim.
    Every one-hot/multiply instruction then runs in the 2x DVE mode.
  - DVE is then the sole bottleneck (~96% busy). For a fraction of blocks
    the value-weighted q one-hot (VQ) is built by the GPSIMD engine's
    local_scatter instruction instead (scatter channel values to
    j*64+q against a zeroed destination), which skips both the q one-hot
    compare and the multiply on DVE for those blocks and overlaps with
    DVE building the r one-hots.
"""

import sys

sys.path.insert(0, "/opt/trn_rl_repo")

import numpy as np
import ml_dtypes

from concourse import bass, bacc, mybir, tile
from concourse.bass_utils import run_bass_kernel_spmd

N_CORES = 8
TOTAL = 8388608
SHARD = TOTAL // N_CORES  # 1048576
BATCH = 8192
P = 128
F32 = mybir.dt.float32
BF16 = mybir.dt.bfloat16
NP_BF16 = ml_dtypes.bfloat16

K_SHARP = 10.0
EPS = 1e-10

S = 64   # element-groups per one-hot build block
SS = 16  # groups per GPSIMD local_scatter call (num_elems = SS*64 <= 2047)


def build_nc(to_count, ti):
    """Build + compile the SPMD bass program. Per core handles
    to_count * 128 * ti elements."""
    shard = to_count * P * ti
    assert shard == SHARD
    nc = bacc.Bacc(
        "TRN2",
        debug=False,
        target_bir_lowering=False,
        num_devices=N_CORES,
    )
    v0_in = nc.dram_tensor("v0", [shard], F32, kind="ExternalInput")
    v1_in = nc.dram_tensor("v1", [shard], F32, kind="ExternalInput")
    v2_in = nc.dram_tensor("v2", [shard], F32, kind="ExternalInput")
    r_in = nc.dram_tensor("ridx2", [2 * shard], BF16, kind="ExternalInput")
    q_in = nc.dram_tensor("qidx2", [2 * shard], BF16, kind="ExternalInput")
    s_in = nc.dram_tensor("spos", [shard], mybir.dt.int16, kind="ExternalInput")
    io128_in = nc.dram_tensor("iota128", [P, 128], BF16, kind="ExternalInput")
    io64_in = nc.dram_tensor("iota64", [P, 64], BF16, kind="ExternalInput")
    out_logits = nc.dram_tensor("logits", [2, BATCH], F32, kind="ExternalOutput")

    with tile.TileContext(nc) as tc:
        _kernel_body(
            tc, to_count, ti,
            v0_in, v1_in, v2_in, r_in, q_in, s_in, io128_in, io64_in,
            out_logits,
        )
    nc.compile()
    return nc


def _kernel_body(tc, to_count, ti,
                 v0_in, v1_in, v2_in, r_in, q_in, s_in, io128_in, io64_in,
                 out_logits):
    nc = tc.nc
    add = mybir.AluOpType.add
    is_equal = mybir.AluOpType.is_equal
    mult = mybir.AluOpType.mult
    AF = mybir.ActivationFunctionType

    v0v = v0_in.ap().rearrange("(o p f) -> o p f", p=P, f=ti)
    v1v = v1_in.ap().rearrange("(o p f) -> o p f", p=P, f=ti)
    v2v = v2_in.ap().rearrange("(o p f) -> o p f", p=P, f=ti)
    rv = r_in.ap().rearrange("(o p f) -> o p f", p=P, f=2 * ti)
    qv = q_in.ap().rearrange("(o p f) -> o p f", p=P, f=2 * ti)
    sv = s_in.ap().rearrange("(o p f) -> o p f", p=P, f=ti)

    assert ti % S == 0
    nb = ti // S
    nblocks = to_count * nb

    with (
        tc.tile_pool(name="const", bufs=1) as cpool,
        tc.tile_pool(name="data", bufs=2) as dpool,
        tc.tile_pool(name="bpool", bufs=3) as bpool,
        tc.tile_pool(name="ohq", bufs=3) as opool,
        tc.tile_pool(name="vq", bufs=3) as vpool,
        tc.tile_pool(name="psum", bufs=1, space="PSUM") as ppool,
        tc.tile_pool(name="epi", bufs=1) as epool,
        tc.tile_pool(name="dram", bufs=1, space="DRAM") as drampool,
    ):
        io128 = cpool.tile([P, 128], BF16)
        nc.sync.dma_start(io128[:], io128_in.ap())
        io64 = cpool.tile([P, 64], BF16)
        nc.sync.dma_start(io64[:], io64_in.ap())
        # pair views of the iota rows: address k = 2*kh + kl
        io128p = io128[:].rearrange("p (kh kl) -> p kh kl", kl=2)
        io64p = io64[:].rearrange("p (kh kl) -> p kh kl", kl=2)

        acc_e = ppool.tile([P, 128], F32, tag="acc_e")
        acc_o = ppool.tile([P, 128], F32, tag="acc_o")

        for to in range(to_count):
            vt0 = dpool.tile([P, ti], F32, tag="vt0")
            nc.sync.dma_start(vt0[:], v0v[to])
            vt1 = dpool.tile([P, ti], F32, tag="vt1")
            nc.sync.dma_start(vt1[:], v1v[to])
            vt2 = dpool.tile([P, ti], F32, tag="vt2")
            nc.sync.dma_start(vt2[:], v2v[to])
            rt2 = dpool.tile([P, 2 * ti], BF16, tag="rt2")
            nc.sync.dma_start(rt2[:], rv[to])
            qt2 = dpool.tile([P, 2 * ti], BF16, tag="qt2")
            nc.sync.dma_start(qt2[:], qv[to])
            spt = dpool.tile([P, ti], mybir.dt.int16, tag="spt")
            nc.sync.dma_start(spt[:], sv[to])

            # channel values duplicated pairwise: cpair2[p, c, t, d] = c_c[p, t]
            cpair2 = dpool.tile([P, 2 * 2 * ti], BF16, tag="cpair2")
            cp4 = cpair2[:].rearrange("p (c t d) -> p c t d", c=2, d=2)
            vt0b = (
                vt0[:].rearrange("p (t o) -> p t o", o=1).to_broadcast([P, ti, 2])
            )
            nc.vector.tensor_copy(cp4[:, 0], vt0b)
            vt1b = (
                vt1[:].rearrange("p (t o) -> p t o", o=1).to_broadcast([P, ti, 2])
            )
            vt2b = (
                vt2[:].rearrange("p (t o) -> p t o", o=1).to_broadcast([P, ti, 2])
            )
            nc.vector.tensor_tensor(cp4[:, 1], vt1b, vt2b, add)

            # contiguous (non-duplicated) channel values for local_scatter
            cflat = dpool.tile([P, 2 * ti], BF16, tag="cflat")
            cf2 = cflat[:].rearrange("p (c t) -> p c t", c=2)
            nc.vector.tensor_copy(cf2[:, 0], vt0[:])
            nc.vector.tensor_tensor(cf2[:, 1], vt1[:], vt2[:], add)

            rt2v = rt2[:].rearrange("p (t d) -> p t d", d=2)
            qt2v = qt2[:].rearrange("p (t d) -> p t d", d=2)

            for b in range(nb):
                sl = slice(b * S, (b + 1) * S)
                blk = to * nb + b

                # r one-hot, normal layout: B[p, j, k] = (r[p, j] == k).
                # All operands end in the packed duplicate-pair dim [1, 2]:
                #   in0 = rt2 pairs broadcast over kh (middle stride 0)
                #   in1 = io128 pair view broadcast over j
                B_all = bpool.tile([P, S * 128], BF16, tag="B")
                B4 = B_all[:].rearrange("p (j kh kl) -> p j kh kl", kh=64, kl=2)
                rb = (
                    rt2v[:, sl]
                    .rearrange("p j (o d) -> p j o d", o=1)
                    .to_broadcast([P, S, 64, 2])
                )
                iob = (
                    io128p[:]
                    .rearrange("p (o kh) kl -> p o kh kl", o=1)
                    .to_broadcast([P, S, 64, 2])
                )
                nc.vector.tensor_tensor(B4, rb, iob, is_equal)

                VQ_all = vpool.tile([P, 2 * S * 64], BF16, tag="VQ")
                VQ4 = VQ_all[:].rearrange("p (c j k) -> p c j k", c=2, k=64)
                # Homogeneous split per block: GPSIMD local_scatter builds
                # VQ for groups j in [0, S/2) (both channels, the scatter
                # zeroes its own destination); DVE builds the q one-hot +
                # multiply for j in [S/2, S). Balances ~8.8us GPSIMD vs
                # ~7.9us DVE per block so every block pipelines identically.
                HGP = S // 2
                for ch in range(2):
                    for m in range(HGP // SS):
                        jlo = b * S + m * SS
                        nc.gpsimd.local_scatter(
                            VQ4[:, ch, m * SS:(m + 1) * SS],
                            cf2[:, ch, jlo:jlo + SS],
                            spt[:, jlo:jlo + SS],
                            channels=P,
                            num_elems=SS * 64,
                            num_idxs=SS,
                        )

                sl_d = slice(b * S + HGP, (b + 1) * S)
                SD = S - HGP
                OHQ_all = opool.tile([P, SD * 64], BF16, tag="OHQ")
                OHQ4 = OHQ_all[:].rearrange(
                    "p (j kh kl) -> p j kh kl", kh=32, kl=2
                )
                qb = (
                    qt2v[:, sl_d]
                    .rearrange("p j (o d) -> p j o d", o=1)
                    .to_broadcast([P, SD, 32, 2])
                )
                ioqb = (
                    io64p[:]
                    .rearrange("p (o kh) kl -> p o kh kl", o=1)
                    .to_broadcast([P, SD, 32, 2])
                )
                nc.vector.tensor_tensor(OHQ4, qb, ioqb, is_equal)

                ohq_pair = OHQ_all[:].rearrange(
                    "p (j kh kl) -> p j kh kl", kh=32, kl=2
                )
                for ch in range(2):
                    cb = (
                        cp4[:, ch, sl_d]
                        .rearrange("p j (o d) -> p j o d", o=1)
                        .to_broadcast([P, SD, 32, 2])
                    )
                    vq_ch = VQ4[:, ch, HGP:S].rearrange(
                        "p j (kh kl) -> p j kh kl", kl=2
                    )
                    nc.vector.tensor_tensor(vq_ch, ohq_pair, cb, mult)

                pacc = acc_e if blk % 2 == 0 else acc_o
                Bj = B_all[:].rearrange("p (j k) -> p j k", k=128)
                VQr = VQ_all[:].rearrange("p (c j k) -> p j c k", c=2, k=64)
                for j in range(S):
                    first = blk < 2 and j == 0
                    last = blk >= nblocks - 2 and j == S - 1
                    nc.tensor.matmul(
                        pacc[:],
                        lhsT=Bj[:, j],
                        rhs=VQr[:, j],
                        start=first,
                        stop=last,
                    )

        # Drain both PSUM accumulators (DVE may read only one PSUM input)
        s_sb = epool.tile([P, 128], F32)
        s_o = epool.tile([P, 128], F32)
        nc.vector.tensor_copy(s_o[:], acc_o[:])
        nc.vector.tensor_tensor(s_sb[:], s_o[:], acc_e[:], add)

        # AllReduce partials across the 8 cores (DRAM bounce buffers)
        din = drampool.tile([P, 128], F32)
        dout = drampool.tile([P, 128], F32)
        nc.gpsimd.dma_start(din[:], s_sb[:])
        nc.gpsimd.collective_compute(
            "AllReduce",
            add,
            replica_groups=[list(range(N_CORES))],
            ins=[din.opt()],
            outs=[dout.opt()],
        )
        sf = epool.tile([P, 128], F32)
        nc.gpsimd.dma_start(sf[:], dout[:])

        # Epilogue: out_c = log(sigmoid(z) + eps), z = -10*s + bias_c.
        # sigmoid computed exactly as 1/(1 + exp(-z)) (ACT exp table +
        # accurate DVE reciprocal); -z clamped at 88 to avoid exp
        # overflow (beyond that sigmoid+eps == eps in fp32 anyway).
        beps = epool.tile([P, 1], F32)
        nc.vector.memset(beps[:], EPS)

        def logsig(out_ap, s_ap, zbias):
            mz = epool.tile([P, 64], F32, tag="mz")
            nc.vector.tensor_scalar(mz[:], s_ap, K_SHARP, -zbias,
                                    mybir.AluOpType.mult, mybir.AluOpType.add)
            nc.vector.tensor_scalar(mz[:], mz[:], 88.0, None,
                                    mybir.AluOpType.min)
            w = epool.tile([P, 64], F32, tag="w")
            nc.scalar.activation(w[:], mz[:], AF.Exp, bias=0.0, scale=1.0)
            nc.vector.tensor_scalar(w[:], w[:], 1.0, None,
                                    mybir.AluOpType.add)
            r = epool.tile([P, 64], F32, tag="r")
            nc.vector.reciprocal(r[:], w[:])
            nc.scalar.activation(out_ap, r[:], AF.Ln, bias=beps[:], scale=1.0)

        o1 = epool.tile([P, 64], F32)
        logsig(o1[:], sf[:, 64:128], K_SHARP)
        o0 = epool.tile([P, 64], F32)
        logsig(o0[:], sf[:, 0:64], 5.0 * K_SHARP)

        ol = out_logits.ap().rearrange("w (p t) -> w p t", p=P, t=BATCH // P)
        nc.sync.dma_start(ol[0], o1[:])
        nc.sync.dma_start(ol[1], o0[:])


_NC_CACHE = {}


def _get_nc(to_count, ti):
    key = (to_count, ti)
    if key not in _NC_CACHE:
        _NC_CACHE[key] = build_nc(to_count, ti)
    return _NC_CACHE[key]


def make_in_maps(sub_logits, original_indices, to_count, ti):
    shard = to_count * P * ti
    idx = np.asarray(original_indices).astype(np.int32)
    v = np.asarray(sub_logits, dtype=np.float32)
    # duplicated pairwise so DVE broadcast reads end in a packed [1, 2] dim
    r2 = np.repeat((idx >> 6).astype(NP_BF16), 2)
    q2 = np.repeat((idx & 63).astype(NP_BF16), 2)
    v0 = np.ascontiguousarray(v[:, 0]).reshape(N_CORES, shard)
    v1 = np.ascontiguousarray(v[:, 1]).reshape(N_CORES, shard)
    v2 = np.ascontiguousarray(v[:, 2]).reshape(N_CORES, shard)
    rs = r2.reshape(N_CORES, 2 * shard)
    qs = q2.reshape(N_CORES, 2 * shard)
    io128 = np.ascontiguousarray(
        np.broadcast_to(np.arange(128, dtype=NP_BF16), (P, 128))
    )
    io64 = np.ascontiguousarray(
        np.broadcast_to(np.arange(64, dtype=NP_BF16), (P, 64))
    )
    # local_scatter positions: element at column t scatters to
    # (t % SS) * 64 + q within its SS-group window
    tmod = ((np.arange(ti, dtype=np.int16) % SS) * 64)
    spos = ((idx & 63).astype(np.int16).reshape(N_CORES, to_count, P, ti)
            + tmod[None, None, None, :])
    spos = np.ascontiguousarray(spos).reshape(N_CORES, shard)
    return [
        {
            "v0": v0[c],
            "v1": v1[c],
            "v2": v2[c],
            "ridx2": rs[c],
            "qidx2": qs[c],
            "spos": spos[c],
            "iota128": io128,
            "iota64": io64,
        }
        for c in range(N_CORES)
    ]


def kernel(sub_logits, original_indices, batch_size=None, _trace=False):
    to_count, ti = 16, 512
    nc = _get_nc(to_count, ti)
    in_maps = make_in_maps(sub_logits, original_indices, to_count, ti)
    res = run_bass_kernel_spmd(
        nc, in_maps, core_ids=list(range(N_CORES)), trace=_trace
    )
    logits = res.results[0]["logits"]
    out = np.stack([logits[0], logits[1]], axis=1).astype(np.float32)
    if _trace:
        kernel._last_results = res
    return out
